# revision 12
# baseline (speedup 1.0000x reference)
"""Trainium2 8-core kernel for the MoE transformer block (nn_MoEBlock_11579231830574).

SPMD over 8 cores; core c owns attention heads {2c,2c+1} and expert c.
  A. attention head-parallel in fp32r (full-speed ~tf32 matmuls); RMSNorm1 folded
     into host-premultiplied weights + on-device per-token scale; causal softmax
     without max subtraction; Wo partial row-major + x/8 -> ReduceScatter: each
     core owns a 512-token slice of x2 (f32, routing-accurate).
  B. routing locally on the slice: logits via transpose + fp32r matmul, top-2 by
     max/compare, combine weights; AllGather bf16 normalized tokens + cw.
  C. MoE expert-parallel with token compaction (capacity 1152): sparse_gather
     index list (big-filler tail), indirect-DMA row gather, PE transpose, SwiGLU
     in bf16, cw scaling, indirect scatter into zeroed buffer -> ReduceScatter ->
     + residual slice -> per-core 512-token output slice; host concatenates.
"""
import numpy as np
import ml_dtypes

import concourse.bass as bass
import concourse.bacc as bacc
import concourse.tile as tile
from concourse import mybir
from concourse.bass_utils import run_bass_kernel_spmd
from concourse.masks import make_identity

dt = mybir.dt
F32, F32R, BF16, I32, U32 = dt.float32, dt.float32r, dt.bfloat16, dt.int32, dt.uint32
OP = mybir.AluOpType
AF = mybir.ActivationFunctionType

B, S, HID = 2, 2048, 2048
T = B * S
NH, HD = 16, 128
NE = 8
INTER = 4096
EPS = 1e-5
P = 128
TN = 512
KT = HID // P              # 16
TT = T // TN               # 8
CAP = 1152
NST = CAP // P             # 9
SGF = T // 16              # 256
SGFILL = CAP // 16         # 72
SGIN = SGF + SGFILL        # 328
NCORES = 8
SLICE = T // NCORES        # 512

_CACHE = {}


def _build():
    nc = bacc.Bacc("TRN2", target_bir_lowering=False, debug=False, num_devices=NCORES)

    xT_d = nc.dram_tensor("xT", [HID, T], F32, kind="ExternalInput").ap()
    x8_d = nc.dram_tensor("x8", [T, HID], F32, kind="ExternalInput").ap()
    wqT_d = nc.dram_tensor("wqT", [HID, 2 * HD], F32, kind="ExternalInput").ap()
    wkT_d = nc.dram_tensor("wkT", [HID, 2 * HD], F32, kind="ExternalInput").ap()
    wvT_d = nc.dram_tensor("wvT", [HID, 2 * HD], F32, kind="ExternalInput").ap()
    woT_d = nc.dram_tensor("woT", [2 * HD, HID], F32, kind="ExternalInput").ap()
    wrT_d = nc.dram_tensor("wrT", [HID, NE], F32, kind="ExternalInput").ap()
    wgT_d = nc.dram_tensor("wgT", [HID, INTER], BF16, kind="ExternalInput").ap()
    wuT_d = nc.dram_tensor("wuT", [HID, INTER], BF16, kind="ExternalInput").ap()
    wdT_d = nc.dram_tensor("wdT", [INTER, HID], BF16, kind="ExternalInput").ap()
    mask_d = nc.dram_tensor("maskdiag", [P, P], F32, kind="ExternalInput").ap()
    ones_d = nc.dram_tensor("onesin", [P, P], F32, kind="ExternalInput").ap()
    id_d = nc.dram_tensor("idin", [P, P], F32, kind="ExternalInput").ap()
    iota_d = nc.dram_tensor("iota16", [16, SGIN], F32, kind="ExternalInput").ap()
    sel16_d = nc.dram_tensor("sel16", [16, SGF * NE], F32, kind="ExternalInput").ap()
    sel128_d = nc.dram_tensor("sel128", [P, NE], F32, kind="ExternalInput").ap()
    out_d = nc.dram_tensor("out_slice", [SLICE, HID], F32, kind="ExternalOutput").ap()

    def r32(ap):
        return ap.bitcast(F32R)

    RG = [list(range(NCORES))]
    SC = float(1.0 / np.sqrt(HD))

    with tile.TileContext(nc) as tc:
        with (
            tc.tile_pool(name="const", bufs=1) as pc,
            tc.tile_pool(name="dram", bufs=1, space="DRAM") as dram,
        ):
            ident_f = pc.tile([P, P], F32R, tag="idf")
            nc.sync.dma_start(out=ident_f[:], in_=id_d[:].bitcast(F32R))
            ident_b = pc.tile([P, P], BF16, tag="idb")
            make_identity(nc, ident_b)
            mask_t = pc.tile([P, P], F32R, tag="mask")
            nc.sync.dma_start(out=mask_t[:], in_=mask_d[:].bitcast(F32R))
            ones_c = pc.tile([P, 1], F32R, tag="ones_c")
            nc.sync.dma_start(out=ones_c[:], in_=ones_d[:, 0:1].bitcast(F32R))
            ones_f = pc.tile([P, 1], F32, tag="ones_f")
            nc.vector.memset(ones_f[:], 1.0)
            ones_r = pc.tile([1, P], F32R, tag="ones_r")
            nc.sync.dma_start(out=ones_r[:], in_=ones_d[0:1, :].bitcast(F32R))
            eps_c = pc.tile([P, 1], F32, tag="eps_c")
            nc.vector.memset(eps_c[:], EPS)

            x2part_rm = dram.tile([T, HID], F32)
            x2slice_dr = dram.tile([SLICE, HID], F32)
            xn2slice_dr = dram.tile([SLICE, HID], BF16)
            cwslice_dr = dram.tile([SLICE, NE], F32)
            xn2_rm = dram.tile([T, HID], BF16, addr_space="Shared")
            cw_all = dram.tile([T, NE], F32, addr_space="Shared")
            idx_dr = dram.tile([CAP], I32)
            yrows_dr = dram.tile([CAP, HID], BF16)
            moe_rm = dram.tile([T, HID], BF16)
            moeslice_dr = dram.tile([SLICE, HID], BF16)

            # ================= Phase A: attention =================
            with (
                tc.tile_pool(name="pwq", bufs=3) as pwq,
                tc.tile_pool(name="pwo", bufs=1) as pwo,
                tc.tile_pool(name="px", bufs=17) as px,
                tc.tile_pool(name="pkv", bufs=1) as pkv,
                tc.tile_pool(name="pat", bufs=3) as pat,
                tc.tile_pool(name="psA", bufs=1, space="PSUM") as psA,
            ):
                wo_t = []
                for ct in range(2):
                    a = pwo.tile([P, HID], F32R, tag=f"wo{ct}")
                    nc.sync.dma_start(out=a[:], in_=woT_d[ct * P:(ct + 1) * P, :].bitcast(F32R))
                    wo_t.append(a)

                kT_sb = [pkv.tile([P, T], F32R, tag=f"kT{h}", name=f"kT{h}") for h in range(2)]
                v_sb = [pkv.tile([P, 2 * HD], F32R, tag=f"v{st}", name=f"v{st}") for st in range(T // P)]

                for tt in range(TT):
                    t0 = tt * TN
                    b = tt // (TT // B)
                    xt = []
                    for kt in range(KT):
                        a = px.tile([P, TN], F32R, tag="xt")
                        nc.sync.dma_start(out=a[:], in_=xT_d[kt * P:(kt + 1) * P, t0:t0 + TN].bitcast(F32R))
                        xt.append(a)
                    # sumsq -> r row -> broadcast + per-token columns
                    ssq = psA.tile([1, TN], F32, tag="b0")
                    for kt in range(KT):
                        sq = pat.tile([P, TN], F32R, tag="sq")
                        nc.scalar.square(sq[:], xt[kt][:])
                        nc.tensor.matmul(ssq[:], r32(ones_c[:]), r32(sq[:]),
                                         start=(kt == 0), stop=(kt == KT - 1))
                    rrow = pat.tile([1, TN], F32R, tag="rrow")
                    nc.scalar.activation(rrow[:], ssq[:], AF.Sqrt, bias=eps_c[0:1, 0:1], scale=1.0 / HID)
                    with nc.allow_low_precision(reason="f32r rms"):
                        nc.vector.reciprocal(rrow[:], rrow[:])
                    rbc_ps = psA.tile([P, TN], F32, tag="b1")
                    nc.tensor.matmul(rbc_ps[:], r32(ones_r[:]), r32(rrow[:]), start=True, stop=True)
                    rbc = pat.tile([P, TN], F32R, tag="rbcs")
                    nc.vector.tensor_copy(rbc[:], rbc_ps[:])
                    r1c = pat.tile([P, TN // P], F32, tag="r1c")
                    for sub in range(TN // P):
                        tp = psA.tile([P, P], F32R, tag="b2")
                        nc.tensor.transpose(tp[:], rbc[:, sub * P:(sub + 1) * P], ident_f[:])
                        nc.vector.tensor_copy(r1c[:, sub:sub + 1], tp[:, 0:1])

                    # q/k for both heads
                    pq = [psA.tile([P, TN], F32, tag=f"b{4+h}", name=f"pq{h}_{tt}") for h in range(2)]
                    pk = [psA.tile([P, TN], F32, tag=f"b{6+h}", name=f"pk{h}_{tt}") for h in range(2)]
                    for kt in range(KT):
                        wq = pwq.tile([P, 2 * HD], F32R, tag="wq")
                        nc.sync.dma_start(out=wq[:], in_=wqT_d[kt * P:(kt + 1) * P, :].bitcast(F32R))
                        wk = pwq.tile([P, 2 * HD], F32R, tag="wk")
                        nc.sync.dma_start(out=wk[:], in_=wkT_d[kt * P:(kt + 1) * P, :].bitcast(F32R))
                        for h in range(2):
                            nc.tensor.matmul(pq[h][:], r32(wq[:, h * HD:(h + 1) * HD]), r32(xt[kt][:]),
                                             start=(kt == 0), stop=(kt == KT - 1))
                            nc.tensor.matmul(pk[h][:], r32(wk[:, h * HD:(h + 1) * HD]), r32(xt[kt][:]),
                                             start=(kt == 0), stop=(kt == KT - 1))
                    q_t = []
                    for h in range(2):
                        qh = pat.tile([P, TN], F32R, tag="qh")
                        nc.vector.scalar_tensor_tensor(out=qh[:], in0=pq[h][:], scalar=SC, in1=rbc[:],
                                                       op0=OP.mult, op1=OP.mult)
                        q_t.append(qh)
                        nc.vector.tensor_mul(kT_sb[h][:, t0:t0 + TN], pk[h][:], rbc[:])
                    # v rows
                    pv = [psA.tile([P, 2 * HD], F32, tag=f"b{sub}", name=f"pv{sub}_{tt}") for sub in range(TN // P)]
                    for kt in range(KT):
                        wv = pwq.tile([P, 2 * HD], F32R, tag="wv")
                        nc.sync.dma_start(out=wv[:], in_=wvT_d[kt * P:(kt + 1) * P, :].bitcast(F32R))
                        for sub in range(TN // P):
                            nc.tensor.matmul(pv[sub][:], r32(xt[kt][:, sub * P:(sub + 1) * P]), r32(wv[:]),
                                             start=(kt == 0), stop=(kt == KT - 1))
                    for sub in range(TN // P):
                        st_i = tt * (TN // P) + sub
                        nc.vector.tensor_scalar(out=v_sb[st_i][:], in0=pv[sub][:],
                                                scalar1=r1c[:, sub:sub + 1], scalar2=None, op0=OP.mult)

                    # causal attention for this q chunk
                    bq0 = t0 - b * S
                    nkv = (bq0 + TN) // P
                    hT_tiles = []
                    for h in range(2):
                        ht_ps = psA.tile([P, TN], F32, tag="b4", name=f"ht_{tt}_{h}")
                        den_ps = psA.tile([1, TN], F32, tag="b5", name=f"den_{tt}_{h}")
                        for kv in range(nkv):
                            st_ps = psA.tile([P, TN], F32, tag=f"b{2 + kv % 2}", name=f"st_{tt}_{h}_{kv}")
                            nc.tensor.matmul(st_ps[:],
                                             r32(kT_sb[h][:, b * S + kv * P: b * S + (kv + 1) * P]),
                                             r32(q_t[h][:]), start=True, stop=True)
                            pt = pat.tile([P, TN], F32R, tag="pt")
                            nc.scalar.activation(pt[:], st_ps[:], AF.Exp)
                            m = kv - (bq0 // P)
                            if m >= 0:
                                if m > 0:
                                    nc.vector.tensor_scalar(out=pt[:, 0:m * P], in0=pt[:, 0:m * P],
                                                            scalar1=0.0, scalar2=None, op0=OP.mult)
                                nc.vector.tensor_mul(pt[:, m * P:(m + 1) * P],
                                                     pt[:, m * P:(m + 1) * P], mask_t[:])
                            nc.tensor.matmul(den_ps[:], r32(ones_c[:]), r32(pt[:]),
                                             start=(kv == 0), stop=(kv == nkv - 1))
                            nc.tensor.matmul(ht_ps[:],
                                             r32(v_sb[(b * S) // P + kv][:, h * HD:(h + 1) * HD]),
                                             r32(pt[:]), start=(kv == 0), stop=(kv == nkv - 1))
                        dinv = pat.tile([1, TN], F32R, tag="dinv")
                        with nc.allow_low_precision(reason="f32r den"):
                            nc.vector.reciprocal(dinv[:], den_ps[:])
                        dbc_ps = psA.tile([P, TN], F32, tag="b6", name=f"dbc_{tt}_{h}")
                        nc.tensor.matmul(dbc_ps[:], r32(ones_r[:]), r32(dinv[:]), start=True, stop=True)
                        dbc = pat.tile([P, TN], F32, tag="dbcs")
                        nc.vector.tensor_copy(dbc[:], dbc_ps[:])
                        hT = pat.tile([P, TN], F32R, tag="hT")
                        nc.vector.tensor_mul(hT[:], ht_ps[:], dbc[:])
                        hT_tiles.append(hT)

                    # o_part rows + x/8
                    for sub in range(TN // P):
                        rt0 = t0 + sub * P
                        for hc in range(HID // TN):
                            po = psA.tile([P, TN], F32, tag=f"b{7 if (sub * (HID // TN) + hc) % 2 == 0 else 0}", name=f"po_{tt}_{sub}_{hc}")
                            for ct in range(2):
                                nc.tensor.matmul(po[:], r32(hT_tiles[ct][:, sub * P:(sub + 1) * P]),
                                                 r32(wo_t[ct][:, hc * TN:(hc + 1) * TN]),
                                                 start=(ct == 0), stop=(ct == 1))
                            x8t = pat.tile([P, TN], F32, tag="x8t")
                            nc.sync.dma_start(out=x8t[:], in_=x8_d[rt0:rt0 + P, hc * TN:(hc + 1) * TN])
                            osb = pat.tile([P, TN], F32, tag="osb")
                            nc.vector.tensor_add(osb[:], po[:], x8t[:])
                            nc.sync.dma_start(out=x2part_rm[rt0:rt0 + P, hc * TN:(hc + 1) * TN],
                                              in_=osb[:])

            nc.gpsimd.collective_compute(
                "ReduceScatter", OP.add, replica_groups=RG,
                ins=[x2part_rm.opt()], outs=[x2slice_dr.opt()],
            )

            # ================= Phase B: routing =================
            with (
                tc.tile_pool(name="pb", bufs=1) as pb,
                tc.tile_pool(name="pbt", bufs=3) as pbt,
                tc.tile_pool(name="psB", bufs=1, space="PSUM") as psB,
            ):
                wr_t = []
                for kt in range(KT):
                    a = pb.tile([P, NE], F32R, tag=f"wr{kt}")
                    nc.sync.dma_start(out=a[:], in_=wrT_d[kt * P:(kt + 1) * P, :].bitcast(F32R))
                    wr_t.append(a)
                x2s = []
                for sub in range(SLICE // P):
                    a = pb.tile([P, HID], F32R, tag=f"x2s{sub}")
                    nc.sync.dma_start(out=a[:], in_=x2slice_dr[sub * P:(sub + 1) * P, :].bitcast(F32R))
                    x2s.append(a)
                x2T = [pb.tile([P, SLICE], F32R, tag=f"x2T{kt}", name=f"x2T{kt}") for kt in range(KT)]
                for sub in range(SLICE // P):
                    for kt in range(KT):
                        tp = psB.tile([P, P], F32R, tag=f"b{kt % 4}", name=f"tpB_{sub}_{kt}")
                        nc.tensor.transpose(tp[:], x2s[sub][:, kt * P:(kt + 1) * P], ident_f[:])
                        nc.vector.tensor_copy(x2T[kt][:, sub * P:(sub + 1) * P], tp[:])
                for sub in range(SLICE // P):
                    sqv = pbt.tile([P, HID], F32, tag="sqv")
                    nc.vector.tensor_mul(sqv[:], x2s[sub][:], x2s[sub][:])
                    ssq = pbt.tile([P, 1], F32, tag="ssq2")
                    nc.vector.tensor_reduce(ssq[:], sqv[:], axis=mybir.AxisListType.X, op=OP.add)
                    r2 = pbt.tile([P, 1], F32, tag="r2")
                    nc.scalar.activation(r2[:], ssq[:], AF.Sqrt, bias=eps_c[:, 0:1], scale=1.0 / HID)
                    nc.vector.reciprocal(r2[:], r2[:])
                    xn2b = pbt.tile([P, HID], BF16, tag="xn2b")
                    nc.vector.tensor_scalar(out=xn2b[:], in0=x2s[sub][:], scalar1=r2[:, 0:1],
                                            scalar2=None, op0=OP.mult)
                    nc.sync.dma_start(out=xn2slice_dr[sub * P:(sub + 1) * P, :], in_=xn2b[:])
                    pl = psB.tile([P, NE], F32, tag="b4", name=f"pl_{sub}")
                    for kt in range(KT):
                        nc.tensor.matmul(pl[:], r32(x2T[kt][:, sub * P:(sub + 1) * P]), r32(wr_t[kt][:]),
                                         start=(kt == 0), stop=(kt == KT - 1))
                    lg = pbt.tile([P, NE], F32, tag="lg")
                    nc.vector.tensor_scalar(out=lg[:], in0=pl[:], scalar1=r2[:, 0:1],
                                            scalar2=None, op0=OP.mult)
                    m1 = pbt.tile([P, 1], F32, tag="m1")
                    nc.vector.tensor_reduce(m1[:], lg[:], axis=mybir.AxisListType.X, op=OP.max)
                    eq1 = pbt.tile([P, NE], F32, tag="eq1")
                    nc.vector.tensor_scalar(out=eq1[:], in0=lg[:], scalar1=m1[:, 0:1], scalar2=None,
                                            op0=OP.is_equal)
                    msk = pbt.tile([P, NE], F32, tag="msk")
                    nc.vector.scalar_tensor_tensor(out=msk[:], in0=eq1[:], scalar=-1e30, in1=lg[:],
                                                   op0=OP.mult, op1=OP.add)
                    m2 = pbt.tile([P, 1], F32, tag="m2")
                    nc.vector.tensor_reduce(m2[:], msk[:], axis=mybir.AxisListType.X, op=OP.max)
                    eq2 = pbt.tile([P, NE], F32, tag="eq2")
                    nc.vector.tensor_scalar(out=eq2[:], in0=msk[:], scalar1=m2[:, 0:1], scalar2=None,
                                            op0=OP.is_equal)
                    d12 = pbt.tile([P, 1], F32, tag="d12")
                    nc.vector.tensor_sub(d12[:], m2[:], m1[:])
                    p2 = pbt.tile([P, 1], F32, tag="p2")
                    nc.scalar.activation(p2[:], d12[:], AF.Sigmoid)
                    p1 = pbt.tile([P, 1], F32, tag="p1")
                    nc.vector.scalar_tensor_tensor(out=p1[:], in0=p2[:], scalar=-1.0, in1=ones_c[:, 0:1],
                                                   op0=OP.mult, op1=OP.add)
                    cw1 = pbt.tile([P, NE], F32, tag="cw1")
                    nc.vector.tensor_scalar(out=cw1[:], in0=eq1[:], scalar1=p1[:, 0:1], scalar2=None,
                                            op0=OP.mult)
                    cw2 = pbt.tile([P, NE], F32, tag="cw2")
                    nc.vector.tensor_scalar(out=cw2[:], in0=eq2[:], scalar1=p2[:, 0:1], scalar2=None,
                                            op0=OP.mult)
                    cwt = pbt.tile([P, NE], F32, tag="cwt")
                    nc.vector.tensor_add(cwt[:], cw1[:], cw2[:])
                    nc.sync.dma_start(out=cwslice_dr[sub * P:(sub + 1) * P, :], in_=cwt[:])

            nc.gpsimd.collective_compute(
                "AllGather", OP.bypass, replica_groups=RG,
                ins=[xn2slice_dr.opt()], outs=[xn2_rm.opt()],
            )
            nc.gpsimd.collective_compute(
                "AllGather", OP.bypass, replica_groups=RG,
                ins=[cwslice_dr.opt()], outs=[cw_all.opt()],
            )

            # ================= Phase C: MoE =================
            with (
                tc.tile_pool(name="pcs", bufs=1) as pcs,
                tc.tile_pool(name="pct", bufs=2) as pct,
                tc.tile_pool(name="psC", bufs=1, space="PSUM") as psC,
            ):
                # C1: index list via sparse_gather
                sel16 = pcs.tile([16, SGF * NE], F32, tag="sel16")
                nc.sync.dma_start(out=sel16[:], in_=sel16_d[:])
                sel128 = pcs.tile([P, NE], F32, tag="sel128")
                nc.sync.dma_start(out=sel128[:], in_=sel128_d[:])
                cw8 = pcs.tile([16, SGF * NE], F32, tag="cw8")
                nc.sync.dma_start(out=cw8[:].rearrange("p (f e) -> p f e", e=NE),
                                  in_=cw_all[:].rearrange("(f p) e -> p f e", p=16))
                nc.vector.tensor_mul(cw8[:], cw8[:], sel16[:])
                cwc = pcs.tile([16, SGF], F32, tag="cwc")
                nc.vector.tensor_reduce(cwc[:], cw8[:].rearrange("p (f e) -> p f e", e=NE),
                                        axis=mybir.AxisListType.X, op=OP.add)
                vals = pcs.tile([16, SGIN], F32, tag="vals")
                nc.sync.dma_start(out=vals[:], in_=iota_d[:])
                mm = pcs.tile([16, SGF], F32, tag="mm")
                nc.vector.tensor_scalar(out=mm[:], in0=cwc[:], scalar1=0.0, scalar2=None, op0=OP.is_gt)
                iv = pcs.tile([16, SGF], F32, tag="iv")
                nc.vector.tensor_mul(iv[:], vals[:, 0:SGF], mm[:])
                nc.vector.tensor_add(iv[:], iv[:], mm[:])
                nc.vector.tensor_scalar(out=vals[:, 0:SGF], in0=iv[:], scalar1=1.0, scalar2=None,
                                        op0=OP.subtract)
                sgo = pcs.tile([16, SGIN], F32, tag="sgo")
                sgc = pcs.tile([1, 1], U32, tag="sgc")
                nc.gpsimd.sparse_gather(sgo[:], vals[:], num_found=sgc[:])
                idx_w = pcs.tile([16, SGFILL], I32, tag="idxw")
                nc.vector.tensor_copy(idx_w[:], sgo[:, 0:SGFILL])
                nc.sync.dma_start(out=idx_dr[:].rearrange("(f p) -> p f", p=16), in_=idx_w[:])
                idx128 = pcs.tile([P, NST], I32, tag="idx128")
                nc.sync.dma_start(out=idx128[:], in_=idx_dr[:].rearrange("(g q) -> q g", q=P))

                # C2: gather + transpose
                _cm_pcx = tc.tile_pool(name="pcx", bufs=1)
                pcx = _cm_pcx.__enter__()
                _cm_pw2 = tc.tile_pool(name="pw2", bufs=2)
                pw2 = _cm_pw2.__enter__()
                xcT = [pcx.tile([P, CAP], BF16, tag=f"xcT{kt}", name=f"xcT{kt}") for kt in range(KT)]
                cws = pcs.tile([P, NST], F32, tag="cws")
                for st in range(NST):
                    xc = pct.tile([P, HID], BF16, tag="xc")
                    nc.vector.memset(xc[:], 0.0)
                    nc.gpsimd.indirect_dma_start(
                        out=xc[:], out_offset=None, in_=xn2_rm[:],
                        in_offset=bass.IndirectOffsetOnAxis(ap=idx128[:, st:st + 1], axis=0),
                        bounds_check=T - 1, oob_is_err=False,
                    )
                    cwg = pct.tile([P, NE], F32, tag="cwg")
                    nc.vector.memset(cwg[:], 0.0)
                    nc.gpsimd.indirect_dma_start(
                        out=cwg[:], out_offset=None, in_=cw_all[:],
                        in_offset=bass.IndirectOffsetOnAxis(ap=idx128[:, st:st + 1], axis=0),
                        bounds_check=T - 1, oob_is_err=False,
                    )
                    nc.vector.tensor_mul(cwg[:], cwg[:], sel128[:])
                    nc.vector.tensor_reduce(cws[:, st:st + 1], cwg[:], axis=mybir.AxisListType.X,
                                            op=OP.add)
                    for kt in range(KT):
                        tp = psC.tile([P, P], BF16, tag=f"b{kt % 2}", name=f"tpC_{st}_{kt}")
                        nc.tensor.transpose(tp[:], xc[:, kt * P:(kt + 1) * P], ident_b[:])
                        nc.vector.tensor_copy(xcT[kt][:, st * P:(st + 1) * P], tp[:])

                # C4: g/u + silu
                a_sb = [pcs.tile([P, CAP], BF16, tag=f"a{it}", name=f"a{it}") for it in range(INTER // P)]
                chunks = []
                off = 0
                while off < CAP:
                    n = min(TN, CAP - off)
                    chunks.append((off, n))
                    off += n
                for it in range(INTER // P):
                    wg_t, wu_t = [], []
                    for kt in range(KT):
                        a = pw2.tile([P, P], BF16, tag=f"wg{kt}", name=f"wg{kt}_{it}")
                        nc.sync.dma_start(out=a[:], in_=wgT_d[kt * P:(kt + 1) * P, it * P:(it + 1) * P])
                        wg_t.append(a)
                        a = pw2.tile([P, P], BF16, tag=f"wu{kt}", name=f"wu{kt}_{it}")
                        nc.sync.dma_start(out=a[:], in_=wuT_d[kt * P:(kt + 1) * P, it * P:(it + 1) * P])
                        wu_t.append(a)
                    for (off, n) in chunks:
                        pg = psC.tile([P, TN], F32, tag=f"b{2 + (it + len(chunks)) % 2}", name=f"pg_{it}_{off}")
                        for kt in range(KT):
                            nc.tensor.matmul(pg[:, :n], wg_t[kt][:], xcT[kt][:, off:off + n],
                                             start=(kt == 0), stop=(kt == KT - 1))
                        pu = psC.tile([P, TN], F32, tag=f"b{4 + (it + len(chunks)) % 2}", name=f"pu_{it}_{off}")
                        for kt in range(KT):
                            nc.tensor.matmul(pu[:, :n], wu_t[kt][:], xcT[kt][:, off:off + n],
                                             start=(kt == 0), stop=(kt == KT - 1))
                        sg_ = pct.tile([P, TN], F32, tag="sg")
                        nc.scalar.activation(sg_[:, :n], pg[:, :n], AF.Silu)
                        nc.vector.tensor_mul(a_sb[it][:, off:off + n], sg_[:, :n], pu[:, :n])

                _cm_pw2.__exit__(None, None, None)
                _cm_pcx.__exit__(None, None, None)

                # C5: y = a @ WdT scaled by cw
                _cm_pwd = tc.tile_pool(name="pwd", bufs=34)
                pwd = _cm_pwd.__enter__()
                for hc in range(HID // TN):
                    wd_t = []
                    for it in range(INTER // P):
                        a = pwd.tile([P, TN], BF16, tag="wd")
                        nc.sync.dma_start(out=a[:], in_=wdT_d[it * P:(it + 1) * P, hc * TN:(hc + 1) * TN])
                        wd_t.append(a)
                    for st in range(NST):
                        py = psC.tile([P, TN], F32, tag=f"b{6 + st % 2}", name=f"py_{hc}_{st}")
                        for it in range(INTER // P):
                            nc.tensor.matmul(py[:], a_sb[it][:, st * P:(st + 1) * P], wd_t[it][:],
                                             start=(it == 0), stop=(it == INTER // P - 1))
                        yb = pct.tile([P, TN], BF16, tag="yb")
                        nc.vector.tensor_scalar(out=yb[:], in0=py[:], scalar1=cws[:, st:st + 1],
                                                scalar2=None, op0=OP.mult)
                        nc.sync.dma_start(out=yrows_dr[st * P:(st + 1) * P, hc * TN:(hc + 1) * TN],
                                          in_=yb[:])

                _cm_pwd.__exit__(None, None, None)
                zt = pct.tile([P, HID], BF16, tag="zt")
                nc.vector.memset(zt[:], 0.0)
                for i in range(T // P):
                    nc.sync.dma_start(out=moe_rm[i * P:(i + 1) * P, :], in_=zt[:])
                for st in range(NST):
                    yrow = pct.tile([P, HID], BF16, tag="yrow")
                    nc.sync.dma_start(out=yrow[:], in_=yrows_dr[st * P:(st + 1) * P, :])
                    nc.gpsimd.indirect_dma_start(
                        out=moe_rm[:],
                        out_offset=bass.IndirectOffsetOnAxis(ap=idx128[:, st:st + 1], axis=0),
                        in_=yrow[:], in_offset=None,
                        bounds_check=T - 1, oob_is_err=False,
                    )

            nc.gpsimd.collective_compute(
                "ReduceScatter", OP.add, replica_groups=RG,
                ins=[moe_rm.opt()], outs=[moeslice_dr.opt()],
            )
            with tc.tile_pool(name="pf", bufs=3) as pf:
                for sub in range(SLICE // P):
                    r1 = pf.tile([P, HID], F32, tag="r1")
                    nc.sync.dma_start(out=r1[:], in_=x2slice_dr[sub * P:(sub + 1) * P, :])
                    m1_ = pf.tile([P, HID], BF16, tag="m1_")
                    nc.sync.dma_start(out=m1_[:], in_=moeslice_dr[sub * P:(sub + 1) * P, :])
                    o1 = pf.tile([P, HID], F32, tag="o1")
                    nc.vector.tensor_add(o1[:], r1[:], m1_[:])
                    nc.sync.dma_start(out=out_d[sub * P:(sub + 1) * P, :], in_=o1[:])

    nc.compile()
    return nc


def _prep_inputs(inputs):
    x = np.asarray(inputs["x"], np.float32).reshape(T, HID)
    Wq = np.asarray(inputs["Wq"], np.float32)
    Wk = np.asarray(inputs["Wk"], np.float32)
    Wv = np.asarray(inputs["Wv"], np.float32)
    Wo = np.asarray(inputs["Wo"], np.float32)
    w1 = np.asarray(inputs["w_ln1"], np.float32)
    w2 = np.asarray(inputs["w_ln2"], np.float32)
    Wr = np.asarray(inputs["Wr"], np.float32)
    Wg = np.asarray(inputs["Wg"], np.float32)
    Wu = np.asarray(inputs["Wu"], np.float32)
    Wd = np.asarray(inputs["Wd"], np.float32)

    xT = np.ascontiguousarray(x.T)
    x8 = (x / 8.0).astype(np.float32)
    mask = np.ascontiguousarray(np.tril(np.ones((P, P), np.float32)).T)  # [kv,q]: kv<=q
    iota = np.full((16, SGIN), 1e9, np.float32)
    t = np.arange(T)
    iota[t % 16, t // 16] = t.astype(np.float32)
    wrT = np.ascontiguousarray((Wr * w2[None, :]).T)

    in_maps = []
    for c in range(NCORES):
        hs = slice(2 * c * HD, 2 * (c + 1) * HD)
        sel = np.zeros(NE, np.float32)
        sel[c] = 1.0
        in_maps.append({
            "xT": xT, "x8": x8,
            "wqT": np.ascontiguousarray((Wq[hs] * w1[None, :]).T),
            "wkT": np.ascontiguousarray((Wk[hs] * w1[None, :]).T),
            "wvT": np.ascontiguousarray((Wv[hs] * w1[None, :]).T),
            "woT": np.ascontiguousarray(Wo[:, hs].T),
            "wrT": wrT,
            "wgT": np.ascontiguousarray((Wg[c] * w2[None, :]).T).astype(ml_dtypes.bfloat16),
            "wuT": np.ascontiguousarray((Wu[c] * w2[None, :]).T).astype(ml_dtypes.bfloat16),
            "wdT": np.ascontiguousarray(Wd[c].T).astype(ml_dtypes.bfloat16),
            "maskdiag": mask,
            "onesin": np.ones((P, P), np.float32),
            "idin": np.eye(P, dtype=np.float32),
            "iota16": iota,
            "sel16": np.tile(sel, (16, SGF)).astype(np.float32),
            "sel128": np.tile(sel, (P, 1)).astype(np.float32),
        })
    return in_maps


def kernel(**inputs):
    if "nc" not in _CACHE:
        _CACHE["nc"] = _build()
    nc = _CACHE["nc"]
    in_maps = _prep_inputs(inputs)
    res = run_bass_kernel_spmd(nc, in_maps, core_ids=list(range(NCORES)),
                               **_CACHE.get("run_kwargs", {}))
    _CACHE["last_results"] = res
    out = np.concatenate([np.asarray(res.results[c]["out_slice"]) for c in range(NCORES)], axis=0)
    return out.reshape(B, S, HID).astype(np.float32)
